# revision 28
# baseline (speedup 1.0000x reference)
"""Trainium2 Bass kernel for grouped-query causal attention (B=2, T=2048, C=1024,
16 q heads / 4 kv heads, RoPE, fused qkv + output projection).

Sharding: 8 cores = (batch b, kv-head h). Each core:
  - projects x -> qT (4 heads), kT, vT with pre-sliced/pre-scaled bf16 weights
    (transposed layout: channels on partitions, T on free dim)
  - applies RoPE (pair-swap via permutation matmul on PE + DVE mul/add)
  - causal attention for its 4 query heads: S^T blocks in bf16, exp without
    max-subtraction (logits are O(7)), post-exp 0/1 causal mask on gpsimd,
    softmax denominators via a ones column appended to V
  - PV: first key-block-pair (keys 0-255) in bf16 (protects small-window
    queries), remaining block-pairs via fp8 DoubleRow matmuls
    (p in e5m2, V in e4m3) -- one matmul per TWO key blocks at 0.5 cyc/row
  - partial output projection y^T = Wf_local^T @ oT (bf16); final bias is
    added on the host after summing the 4 per-h partials per batch.
"""

import sys

sys.path.insert(0, "/opt/trn_rl_repo")

import ml_dtypes
import numpy as np

import concourse.bacc as bacc
import concourse.mybir as mybir
from concourse import tile
from concourse.bass_utils import run_bass_kernel_spmd

B, T, C = 2, 2048, 1024
G, HKV, HS = 4, 4, 64
OUT_DIM = C + 2 * (C // G)
SCALE = 1.0 / np.sqrt(HS)
MAX_PERIOD = 10000.0

F32 = mybir.dt.float32
BF16 = mybir.dt.bfloat16
F8E4 = mybir.dt.float8e4
F8E5 = mybir.dt.float8e5
AF = mybir.ActivationFunctionType
DR = mybir.MatmulPerfMode.DoubleRow

TCH = T // 512  # 4 chunks of 512 along T
NT = T // 128  # 16 tiles of 128 along T


def build_nc():
    nc = bacc.Bacc(None, target_bir_lowering=False)

    xT_d = nc.dram_tensor("xT", [C, T], BF16, kind="ExternalInput")
    w_d = nc.dram_tensor("w_qkv", [C, 384], BF16, kind="ExternalInput")
    bl_d = nc.dram_tensor("b_loc", [128, 3], F32, kind="ExternalInput")
    cos_d = nc.dram_tensor("cosT", [128, T], BF16, kind="ExternalInput")
    sin_d = nc.dram_tensor("sinT", [128, T], BF16, kind="ExternalInput")
    perm_d = nc.dram_tensor("perm", [128, 128], BF16, kind="ExternalInput")
    eye_d = nc.dram_tensor("eye64", [128, 64], BF16, kind="ExternalInput")
    mask_d = nc.dram_tensor("mask01", [128, 2, 256], BF16, kind="ExternalInput")
    wf_d = nc.dram_tensor("wf", [256, 1024], BF16, kind="ExternalInput")
    ones_d = nc.dram_tensor("onesd", [128, 64], BF16, kind="ExternalInput")
    yT_d = nc.dram_tensor("yT", [C, T], BF16, kind="ExternalOutput")

    with tile.TileContext(nc) as tc:
        with (
            tc.tile_pool(name="persist", bufs=1) as pp,
            tc.tile_pool(name="xstream", bufs=10) as spx,
            tc.tile_pool(name="pstream", bufs=6) as spp,
            tc.tile_pool(name="rstream", bufs=3) as spr,
            tc.tile_pool(name="ostream", bufs=3) as spo,
            tc.tile_pool(name="ps_acc", bufs=3, space="PSUM") as psacc,
            tc.tile_pool(name="ps_s", bufs=2, space="PSUM") as pss,
            tc.tile_pool(name="ps_tmp", bufs=1, space="PSUM") as ps,
        ):
            # ---- persistent tiles ----
            w_sb = pp.tile([128, 8, 384], BF16, tag="w", name="w")
            bl_sb = pp.tile([128, 3], F32, tag="bl", name="bl")
            cos_sb = pp.tile([128, T], BF16, tag="cos", name="cos")
            sin_sb = pp.tile([128, T], BF16, tag="sin", name="sin")
            perm_sb = pp.tile([128, 128], BF16, tag="perm", name="perm")
            eye_sb = pp.tile([128, 64], BF16, tag="eye", name="eye")
            mask_sb = pp.tile([128, 2, 256], BF16, tag="mask", name="mask")
            mask8_sb = pp.tile([128, 2, 256], F8E5, tag="mask8", name="mask8")
            wf_sb = pp.tile([128, 2, 1024], BF16, tag="wf", name="wf")
            ones_sb = pp.tile([128, 64], BF16, tag="ones", name="ones")
            qkvT = [pp.tile([128, T], BF16, tag=f"qkvT{m}", name=f"qkvT{m}") for m in range(3)]
            qcat = [pp.tile([64, 2, T], BF16, tag=f"qcat{m}", name=f"qcat{m}") for m in range(2)]
            v8_sb = pp.tile([128, NT, 80], F8E4, tag="v8", name="v8")
            v01_sb = pp.tile([128, 2, 65], BF16, tag="v01", name="v01")
            oT_ab = [pp.tile([128, T], BF16, tag=f"oT{i}", name=f"oT{i}") for i in range(2)]

            nc.sync.dma_start(bl_sb[:], bl_d[:])
            nc.gpsimd.memset(v8_sb[:, :, 64:65], 1.0)
            nc.gpsimd.memset(v01_sb[:, :, 64:65], 1.0)

            # -- emission helpers ------------------------------------------
            def load_x(tc_i):
                tsl = slice(tc_i * 512, (tc_i + 1) * 512)
                xts = []
                for k in range(8):
                    xt = spx.tile([128, 512], BF16, tag="xt", name="xt")
                    nc.sync.dma_start(xt[:], xT_d[k * 128 : (k + 1) * 128, tsl])
                    xts.append(xt)
                return xts

            def proj_tasks(tc_i, xts):
                """Dense background tasks for chunk tc_i's projection+RoPE+vT.
                Ordered so k/v are ready before q heads (attention needs k,v
                first at chunk 0)."""
                tsl = slice(tc_i * 512, (tc_i + 1) * 512)

                def mk_group(mt):
                    def run():
                        pr = ps.tile([128, 512], F32, tag="tmp", name="tmp")
                        for k in range(8):
                            nc.tensor.matmul(
                                pr[:],
                                w_sb[:, k, mt * 128 : (mt + 1) * 128],
                                xts[k][:],
                                start=(k == 0),
                                stop=(k == 7),
                            )
                        nc.vector.tensor_scalar_add(
                            qkvT[mt][:, tsl], pr[:], bl_sb[:, mt : mt + 1]
                        )
                    return run

                def mk_rope_q(mt):
                    def run():
                        tmp = ps.tile([128, 512], F32, tag="tmp", name="tmp")
                        nc.tensor.matmul(
                            tmp[:], perm_sb[:], qkvT[mt][:, tsl], start=True, stop=True
                        )
                        nc.vector.tensor_mul(
                            qkvT[mt][:, tsl], qkvT[mt][:, tsl], cos_sb[:, tsl]
                        )
                        tmpb = spp.tile([128, 2, 512], BF16, tag="p", name="p")
                        nc.vector.tensor_mul(tmpb[:, 0, :], tmp[:], sin_sb[:, tsl])
                        nc.vector.tensor_add(
                            qkvT[mt][:, tsl], qkvT[mt][:, tsl], tmpb[:, 0, :]
                        )
                        nc.sync.dma_start(qcat[mt][:, 0, tsl], qkvT[mt][0:64, tsl])
                        nc.sync.dma_start(qcat[mt][:, 1, tsl], qkvT[mt][64:128, tsl])
                    return run

                def mk_vt(i):
                    def run():
                        tt = tc_i * 4 + i
                        vt = ps.tile([128, 512], BF16, tag="tmp", name="tmp")
                        nc.tensor.transpose(
                            vt[:, 0:64],
                            qkvT[2][64:128, tt * 128 : (tt + 1) * 128],
                            eye_sb[64:128, :],
                        )
                        nc.vector.tensor_copy(v8_sb[:, tt, 0:64], vt[:, 0:64])
                        if tt < 2:
                            nc.vector.tensor_copy(v01_sb[:, tt, 0:64], vt[:, 0:64])
                    return run

                def rope_k():
                    tmp = ps.tile([128, 512], F32, tag="tmp", name="tmp")
                    nc.tensor.matmul(
                        tmp[0:64, :], perm_sb[:, 0:64], qkvT[2][:, tsl],
                        start=True, stop=True,
                    )
                    nc.vector.tensor_mul(
                        qkvT[2][0:64, tsl], qkvT[2][0:64, tsl], cos_sb[0:64, tsl]
                    )
                    tmpb = spp.tile([128, 2, 512], BF16, tag="p", name="p")
                    nc.vector.tensor_mul(
                        tmpb[0:64, 0, :], tmp[0:64, :], sin_sb[0:64, tsl]
                    )
                    nc.vector.tensor_add(
                        qkvT[2][0:64, tsl], qkvT[2][0:64, tsl], tmpb[0:64, 0, :]
                    )

                return [
                    mk_group(2), rope_k,
                    mk_vt(0), mk_vt(1), mk_vt(2), mk_vt(3),
                    mk_group(0), mk_rope_q(0),
                    mk_group(1), mk_rope_q(1),
                ]

            def final_tasks(tc_i):
                tsl = slice(tc_i * 512, (tc_i + 1) * 512)

                def mk(nt):
                    def run():
                        y_ps = ps.tile([128, 512], F32, tag="tmp", name="tmp")
                        for cc in range(2):
                            nc.tensor.matmul(
                                y_ps[:],
                                wf_sb[:, cc, nt * 128 : (nt + 1) * 128],
                                oT_ab[cc][:, tsl],
                                start=(cc == 0),
                                stop=(cc == 1),
                            )
                        y_sb = spo.tile([128, 512], BF16, tag="yout", name="yout")
                        nc.vector.tensor_copy(y_sb[:], y_ps[:])
                        nc.sync.dma_start(yT_d[nt * 128 : (nt + 1) * 128, tsl], y_sb[:])
                    return run

                return [mk(nt) for nt in range(8)]

            # -- prologue --------------------------------------------------
            # group-2 (k/v) weight slice first: the first proj group needs it
            nc.sync.dma_start(
                w_sb[:, :, 256:384],
                w_d[:, 256:384].rearrange("(k p) n -> p k n", p=128),
            )
            xts0 = load_x(0)
            nc.sync.dma_start(
                w_sb[:, :, 0:256],
                w_d[:, 0:256].rearrange("(k p) n -> p k n", p=128),
            )
            nc.sync.dma_start(perm_sb[:], perm_d[:])
            nc.sync.dma_start(cos_sb[:], cos_d[:])
            nc.sync.dma_start(sin_sb[:], sin_d[:])
            nc.sync.dma_start(eye_sb[:], eye_d[:])
            nc.sync.dma_start(mask_sb[:], mask_d[:])
            nc.sync.dma_start(wf_sb[:], wf_d.rearrange("(c p) n -> p c n", p=128))
            nc.sync.dma_start(ones_sb[:], ones_d[:])
            # fp8e5 copy of the 0/1 mask (dtype-matched to the fp8 p tiles)
            nc.vector.tensor_copy(mask8_sb[:], mask_sb[:])
            for t in proj_tasks(0, xts0):
                t()

            # -- main loop -------------------------------------------------
            pending_norm = []  # deferred norm-finish closures

            def mk_norm(g, tci, o_ac):
                """Normalize o_ac rows 0:64 by the denominator in row 64 and
                store into oT_ab (bf16)."""
                tsl = slice(tci * 512, (tci + 1) * 512)
                odd = g % 2 == 1
                sums = spr.tile([128, 512], BF16, tag="rec", name="rec")
                nc.vector.tensor_copy(sums[64:65, :], o_ac[64:65, :])

                def run():
                    bc = ps.tile([128, 512], F32, tag="tmp", name="tmp")
                    nc.tensor.matmul(
                        bc[0:64, :], ones_sb[64:65, 0:64], sums[64:65, :],
                        start=True, stop=True,
                    )
                    bc_sb = spr.tile([64, 512], F32, tag="bcs", name="bcs")
                    nc.vector.reciprocal_approx_fast(out=bc_sb[:], in_=bc[0:64, :])
                    if odd:
                        stg = spr.tile([64, 512], BF16, tag="stg", name="stg")
                        nc.vector.tensor_mul(stg[:], o_ac[0:64, :], bc_sb[:])
                        nc.sync.dma_start(oT_ab[g // 2][64:128, tsl], stg[:])
                    else:
                        nc.vector.tensor_mul(
                            oT_ab[g // 2][0:64, tsl], o_ac[0:64, :], bc_sb[:]
                        )
                return run

            for tci in range(TCH):
                tsl0 = tci * 512
                njp = 2 * tci + 2

                bg = []  # (gate_chunk | None, closure)
                if tci + 1 < TCH:
                    xts = load_x(tci + 1)
                    bg += [(None, t) for t in proj_tasks(tci + 1, xts)]
                if tci >= 1:
                    # final(tci-1) must not be emitted until every norm of
                    # chunk tci-1 has been emitted (it reads their oT writes)
                    bg += [(tci - 1, t) for t in final_tasks(tci - 1)]
                bg_done = 0
                bg_total = len(bg)
                slots = 2 * 2 * 2 * njp  # pairs x h01 x jp, x4 slot increment
                slot = 0

                def norms_pending_upto(c):
                    return any(ch <= c for ch, _ in pending_norm)

                for pair in range(2):
                    qc = qcat[pair]
                    o_acs = [
                        psacc.tile([128, 512], F32, tag="oacc", name="oacc")
                        for _ in range(2)
                    ]
                    DEPTH = 3
                    pvq = []  # (jp, h01, p_tile, win) awaiting PV emission

                    def emit_pv(jp, h01, p_t, win, o_acs=o_acs, njp=njp):
                        if jp == 0:
                            for i in range(2):
                                nc.tensor.matmul(
                                    o_acs[h01][0:65, :],
                                    v01_sb[:, i, 0:65],
                                    p_t[:, i, :],
                                    start=(i == 0),
                                    stop=False,
                                )
                        else:
                            nc.tensor.matmul(
                                o_acs[h01][0:65, win:512],
                                v8_sb[:, 2 * jp : 2 * jp + 2, 0:65],
                                p_t[:, :, win:512],
                                start=False,
                                stop=(jp == njp - 1),
                                perf_mode=DR,
                            )

                    for jp in range(njp):
                        win = 256 if jp == 2 * tci + 1 else 0
                        diag = jp in (2 * tci, 2 * tci + 1)
                        for h01 in range(2):
                            s_ps = pss.tile([128, 2, 512], F32, tag="s", name="s")
                            for i in range(2):
                                blk = 2 * jp + i
                                nc.tensor.matmul(
                                    s_ps[:, i, win:512],
                                    qkvT[2][0:64, blk * 128 : (blk + 1) * 128],
                                    qc[:, h01, tsl0 + win : tsl0 + 512],
                                    start=True,
                                    stop=True,
                                )
                            pdt = BF16 if jp == 0 else F8E5
                            p_t = spp.tile([128, 2, 512], pdt, tag="p", name="p")
                            nc.scalar.activation(
                                p_t[:, :, win:512], s_ps[:, :, win:512], AF.Exp
                            )
                            if diag:
                                nc.gpsimd.tensor_mul(
                                    p_t[:, :, win : win + 256],
                                    p_t[:, :, win : win + 256],
                                    mask_sb[:] if jp == 0 else mask8_sb[:],
                                )
                            pvq.append((jp, h01, p_t, win))
                            if len(pvq) > DEPTH:
                                emit_pv(*pvq.pop(0))
                        if pending_norm:
                            pending_norm.pop(0)[1]()
                        slot += 4
                        due = bg_total * min(slot, slots) // slots
                        while bg_done < due:
                            gate, fn = bg[bg_done]
                            if gate is not None and norms_pending_upto(gate):
                                break
                            fn()
                            bg_done += 1

                    for args in pvq:
                        emit_pv(*args)
                    for h01 in range(2):
                        g = pair * 2 + h01
                        pending_norm.append((tci, mk_norm(g, tci, o_acs[h01])))

                while bg_done < bg_total:
                    gate, fn = bg[bg_done]
                    if gate is not None and norms_pending_upto(gate):
                        # flush enough pending norms to unblock
                        while norms_pending_upto(gate):
                            pending_norm.pop(0)[1]()
                    fn()
                    bg_done += 1

            for _, fn in pending_norm:
                fn()
            for t in final_tasks(TCH - 1):
                t()

    nc.compile()
    return nc


def host_shard(inputs):
    """Build the 8 per-core input maps from full inputs."""
    x = np.ascontiguousarray(np.asarray(inputs["input"], dtype=np.float32))
    W = np.asarray(inputs["W_attn"], dtype=np.float32)
    bb = np.asarray(inputs["b_attn"], dtype=np.float32)
    Wf = np.asarray(inputs["W_final"], dtype=np.float32)

    half = HS // 2
    inv_freq = MAX_PERIOD ** (-np.arange(half, dtype=np.float32) / half)
    ang = np.arange(T, dtype=np.float32)[:, None] * inv_freq  # (T, 32)
    sin_t = np.sin(ang).astype(np.float32)
    cos_t = np.cos(ang).astype(np.float32)
    cosT = np.repeat(cos_t.T, 2, axis=0)  # (64, T): row d -> cos(t*f[d//2])
    sgn = np.where(np.arange(HS) % 2 == 0, -1.0, 1.0).astype(np.float32)
    sinT = np.repeat(sin_t.T, 2, axis=0) * sgn[:, None]
    cos128 = np.ascontiguousarray(np.concatenate([cosT, cosT], axis=0))
    sin128 = np.ascontiguousarray(np.concatenate([sinT, sinT], axis=0))

    perm = np.zeros((128, 128), np.float32)
    idx = np.arange(128)
    perm[idx ^ 1, idx] = 1.0
    eye64 = np.zeros((128, 64), np.float32)
    eye64[64:128, :] = np.eye(64, dtype=np.float32)

    # 0/1 keep-mask for the two diagonal key-blocks of each chunk, applied on
    # the first 256 computed queries of a (block-pair, window):
    #   slot 0 (earlier block):  u<128: keep kk<=u ; u in [128,256): keep all
    #   slot 1 (later block):    u<128: keep none  ; u in [128,256): keep kk<=u-128
    kk = np.arange(128)[:, None]
    u = np.arange(256)[None, :]
    m0 = np.where(u < 128, (kk <= u), True)
    m1 = np.where(u < 128, False, (kk <= u - 128))
    mask01 = np.ascontiguousarray(
        np.stack([m0, m1], axis=1).astype(np.float32)
    )  # (128, 2, 256)

    ones64 = np.ones((128, 64), np.float32)

    in_maps = []
    for cid in range(8):
        b, h = cid // 4, cid % 4
        qcols = np.concatenate(
            [np.arange(g * 256 + h * 64, g * 256 + h * 64 + 64) for g in range(G)]
        )
        kcols = np.arange(1024 + h * 64, 1024 + h * 64 + 64)
        vcols = np.arange(1280 + h * 64, 1280 + h * 64 + 64)
        cols = np.concatenate([qcols, kcols, vcols])
        w_loc = W[:, cols].copy()
        b_loc = bb[cols].copy()
        w_loc[:, :256] *= SCALE
        b_loc[:256] *= SCALE
        b_loc_m = np.ascontiguousarray(b_loc.reshape(3, 128).T)  # (128, 3)

        rows = np.concatenate(
            [np.arange(g * 256 + h * 64, g * 256 + h * 64 + 64) for g in range(G)]
        )
        wf_loc = np.ascontiguousarray(Wf[rows, :])  # (256, 1024)

        in_maps.append(
            {
                "xT": np.ascontiguousarray(x[b].T).astype(ml_dtypes.bfloat16),
                "w_qkv": w_loc.astype(ml_dtypes.bfloat16),
                "b_loc": b_loc_m,
                "cosT": cos128.astype(ml_dtypes.bfloat16),
                "sinT": sin128.astype(ml_dtypes.bfloat16),
                "perm": perm.astype(ml_dtypes.bfloat16),
                "eye64": eye64.astype(ml_dtypes.bfloat16),
                "mask01": mask01.astype(ml_dtypes.bfloat16),
                "wf": wf_loc.astype(ml_dtypes.bfloat16),
                "onesd": ones64.astype(ml_dtypes.bfloat16),
            }
        )
    return in_maps


def host_unshard(results, b_final):
    """Sum the 4 per-h partial yT per batch, add bias, transpose back."""
    out = np.empty((B, T, C), np.float32)
    for b in range(B):
        acc = results[b * 4]["yT"].astype(np.float32)
        for h in range(1, 4):
            acc = acc + results[b * 4 + h]["yT"]
        out[b] = acc.T + b_final[None, :]
    return out


_NC_CACHE = None


def _get_nc():
    global _NC_CACHE
    if _NC_CACHE is None:
        _NC_CACHE = build_nc()
    return _NC_CACHE


def kernel(**inputs):
    nc = _get_nc()
    in_maps = host_shard(inputs)
    res = run_bass_kernel_spmd(nc, in_maps, core_ids=list(range(8)))
    return host_unshard(res.results, np.asarray(inputs["b_final"], dtype=np.float32))


# revision 35
# speedup vs baseline: 1.0467x; 1.0467x over previous
"""Trainium2 Bass kernel for grouped-query causal attention (B=2, T=2048, C=1024,
16 q heads / 4 kv heads, RoPE, fused qkv + output projection).

Sharding: 8 cores = (batch b, kv-head h). Each core:
  - projects x -> qT (4 heads), kT, vT with pre-sliced/pre-scaled bf16 weights
    (transposed layout: channels on partitions, T on free dim)
  - applies RoPE (pair-swap via permutation matmul on PE + DVE mul/add)
  - causal attention for its 4 query heads: S^T blocks in bf16, exp without
    max-subtraction (logits are O(7)), post-exp 0/1 causal mask on gpsimd,
    softmax denominators via a ones column appended to V
  - PV: first key-block-pair (keys 0-255) in bf16 (protects small-window
    queries), remaining block-pairs via fp8 DoubleRow matmuls
    (p in e5m2, V in e4m3) -- one matmul per TWO key blocks at 0.5 cyc/row
  - partial output projection y^T = Wf_local^T @ oT (bf16); final bias is
    added on the host after summing the 4 per-h partials per batch.
"""

import sys

sys.path.insert(0, "/opt/trn_rl_repo")

import ml_dtypes
import numpy as np

import concourse.bacc as bacc
import concourse.mybir as mybir
from concourse import tile
from concourse.bass_utils import run_bass_kernel_spmd

B, T, C = 2, 2048, 1024
G, HKV, HS = 4, 4, 64
OUT_DIM = C + 2 * (C // G)
SCALE = 1.0 / np.sqrt(HS)
MAX_PERIOD = 10000.0

F32 = mybir.dt.float32
BF16 = mybir.dt.bfloat16
F8E4 = mybir.dt.float8e4
F8E5 = mybir.dt.float8e5
AF = mybir.ActivationFunctionType
DR = mybir.MatmulPerfMode.DoubleRow

TCH = T // 512  # 4 chunks of 512 along T
NT = T // 128  # 16 tiles of 128 along T


def build_nc():
    nc = bacc.Bacc(None, target_bir_lowering=False)

    xT_d = nc.dram_tensor("xT", [C, T], BF16, kind="ExternalInput")
    w_d = nc.dram_tensor("w_qkv", [C, 384], BF16, kind="ExternalInput")
    bl_d = nc.dram_tensor("b_loc", [128, 3], F32, kind="ExternalInput")
    cos_d = nc.dram_tensor("cosT", [128, T], BF16, kind="ExternalInput")
    sin_d = nc.dram_tensor("sinT", [128, T], BF16, kind="ExternalInput")
    perm_d = nc.dram_tensor("perm", [128, 128], BF16, kind="ExternalInput")
    eye_d = nc.dram_tensor("eye64", [128, 64], BF16, kind="ExternalInput")
    mask_d = nc.dram_tensor("mask01", [128, 2, 256], BF16, kind="ExternalInput")
    wf_d = nc.dram_tensor("wf", [256, 1024], BF16, kind="ExternalInput")
    ones_d = nc.dram_tensor("onesd", [128, 64], BF16, kind="ExternalInput")
    yT_d = nc.dram_tensor("yT", [C, T], BF16, kind="ExternalOutput")
    # first-contraction-half partial of the last chunk's output projection
    # (runs early, during pair-1 attention; host adds it back in)
    y2_d = nc.dram_tensor("y2T", [C, 512], BF16, kind="ExternalOutput")

    with tile.TileContext(nc) as tc:
        with (
            tc.tile_pool(name="persist", bufs=1) as pp,
            tc.tile_pool(name="xstream", bufs=10) as spx,
            tc.tile_pool(name="pstream", bufs=6) as spp,
            tc.tile_pool(name="rstream", bufs=3) as spr,
            tc.tile_pool(name="ostream", bufs=3) as spo,
            tc.tile_pool(name="ps_acc", bufs=3, space="PSUM") as psacc,
            tc.tile_pool(name="ps_s", bufs=2, space="PSUM") as pss,
            tc.tile_pool(name="ps_tmp", bufs=1, space="PSUM") as ps,
        ):
            # ---- persistent tiles ----
            w_sb = pp.tile([128, 8, 384], BF16, tag="w", name="w")
            bl_sb = pp.tile([128, 3], F32, tag="bl", name="bl")
            cos_sb = pp.tile([128, T], BF16, tag="cos", name="cos")
            sin_sb = pp.tile([128, T], BF16, tag="sin", name="sin")
            perm_sb = pp.tile([128, 128], BF16, tag="perm", name="perm")
            eye_sb = pp.tile([128, 64], BF16, tag="eye", name="eye")
            mask_sb = pp.tile([128, 2, 256], BF16, tag="mask", name="mask")
            mask8_sb = pp.tile([128, 2, 256], F8E5, tag="mask8", name="mask8")
            wf_sb = pp.tile([128, 2, 1024], BF16, tag="wf", name="wf")
            ones_sb = pp.tile([128, 64], BF16, tag="ones", name="ones")
            qkvT = [pp.tile([128, T], BF16, tag=f"qkvT{m}", name=f"qkvT{m}") for m in range(3)]
            qcat = [pp.tile([64, 2, T], BF16, tag=f"qcat{m}", name=f"qcat{m}") for m in range(2)]
            v8_sb = pp.tile([128, NT, 80], F8E4, tag="v8", name="v8")
            v01_sb = pp.tile([128, 2, 65], BF16, tag="v01", name="v01")
            oT_ab = [pp.tile([128, T], BF16, tag=f"oT{i}", name=f"oT{i}") for i in range(2)]

            nc.sync.dma_start(bl_sb[:], bl_d[:])
            nc.gpsimd.memset(v8_sb[:, :, 64:65], 1.0)
            nc.gpsimd.memset(v01_sb[:, :, 64:65], 1.0)

            # -- emission helpers ------------------------------------------
            def load_x(tc_i):
                tsl = slice(tc_i * 512, (tc_i + 1) * 512)
                xts = []
                for k in range(8):
                    xt = spx.tile([128, 512], BF16, tag="xt", name="xt")
                    nc.sync.dma_start(xt[:], xT_d[k * 128 : (k + 1) * 128, tsl])
                    xts.append(xt)
                return xts

            def proj_tasks(tc_i, xts):
                """Dense background tasks for chunk tc_i's projection+RoPE+vT.
                Ordered so k/v are ready before q heads (attention needs k,v
                first at chunk 0)."""
                tsl = slice(tc_i * 512, (tc_i + 1) * 512)

                def mk_group(mt):
                    def run():
                        pr = ps.tile([128, 512], F32, tag="tmp", name="tmp")
                        for k in range(8):
                            nc.tensor.matmul(
                                pr[:],
                                w_sb[:, k, mt * 128 : (mt + 1) * 128],
                                xts[k][:],
                                start=(k == 0),
                                stop=(k == 7),
                            )
                        nc.vector.tensor_scalar_add(
                            qkvT[mt][:, tsl], pr[:], bl_sb[:, mt : mt + 1]
                        )
                    return run

                def mk_rope_q(mt):
                    def run():
                        tmp = ps.tile([128, 512], F32, tag="tmp", name="tmp")
                        nc.tensor.matmul(
                            tmp[:], perm_sb[:], qkvT[mt][:, tsl], start=True, stop=True
                        )
                        nc.vector.tensor_mul(
                            qkvT[mt][:, tsl], qkvT[mt][:, tsl], cos_sb[:, tsl]
                        )
                        tmpb = spp.tile([128, 2, 512], BF16, tag="p", name="p")
                        nc.vector.tensor_mul(tmpb[:, 0, :], tmp[:], sin_sb[:, tsl])
                        nc.vector.tensor_add(
                            qkvT[mt][:, tsl], qkvT[mt][:, tsl], tmpb[:, 0, :]
                        )
                        nc.sync.dma_start(qcat[mt][:, 0, tsl], qkvT[mt][0:64, tsl])
                        nc.sync.dma_start(qcat[mt][:, 1, tsl], qkvT[mt][64:128, tsl])
                    return run

                def mk_vt(i):
                    def run():
                        tt = tc_i * 4 + i
                        vt = ps.tile([128, 512], BF16, tag="tmp", name="tmp")
                        nc.tensor.transpose(
                            vt[:, 0:64],
                            qkvT[2][64:128, tt * 128 : (tt + 1) * 128],
                            eye_sb[64:128, :],
                        )
                        nc.vector.tensor_copy(v8_sb[:, tt, 0:64], vt[:, 0:64])
                        if tt < 2:
                            nc.vector.tensor_copy(v01_sb[:, tt, 0:64], vt[:, 0:64])
                    return run

                def rope_k():
                    tmp = ps.tile([128, 512], F32, tag="tmp", name="tmp")
                    nc.tensor.matmul(
                        tmp[0:64, :], perm_sb[:, 0:64], qkvT[2][:, tsl],
                        start=True, stop=True,
                    )
                    nc.vector.tensor_mul(
                        qkvT[2][0:64, tsl], qkvT[2][0:64, tsl], cos_sb[0:64, tsl]
                    )
                    tmpb = spp.tile([128, 2, 512], BF16, tag="p", name="p")
                    nc.vector.tensor_mul(
                        tmpb[0:64, 0, :], tmp[0:64, :], sin_sb[0:64, tsl]
                    )
                    nc.vector.tensor_add(
                        qkvT[2][0:64, tsl], qkvT[2][0:64, tsl], tmpb[0:64, 0, :]
                    )

                return [
                    mk_group(2), rope_k,
                    mk_vt(0), mk_vt(1), mk_vt(2), mk_vt(3),
                    mk_group(0), mk_rope_q(0),
                    mk_group(1), mk_rope_q(1),
                ]

            def final_tasks(tc_i, ccs=(0, 1), dst=None):
                tsl = slice(tc_i * 512, (tc_i + 1) * 512)
                d = yT_d if dst is None else dst

                def mk(nt):
                    def run():
                        y_ps = ps.tile([128, 512], F32, tag="tmp", name="tmp")
                        for j, cc in enumerate(ccs):
                            nc.tensor.matmul(
                                y_ps[:],
                                wf_sb[:, cc, nt * 128 : (nt + 1) * 128],
                                oT_ab[cc][:, tsl],
                                start=(j == 0),
                                stop=(j == len(ccs) - 1),
                            )
                        y_sb = spo.tile([128, 512], BF16, tag="yout", name="yout")
                        nc.vector.tensor_copy(y_sb[:], y_ps[:])
                        if dst is None:
                            nc.sync.dma_start(d[nt * 128 : (nt + 1) * 128, tsl], y_sb[:])
                        else:
                            nc.sync.dma_start(d[nt * 128 : (nt + 1) * 128, :], y_sb[:])
                    return run

                return [mk(nt) for nt in range(8)]

            # -- prologue --------------------------------------------------
            # interleave per-k weight and x-tile loads so proj matmul k can
            # start as soon as its own (w_k, x_k) pair has landed
            xts0 = []
            for k in range(8):
                nc.sync.dma_start(w_sb[:, k, :], w_d[k * 128 : (k + 1) * 128, :])
                xt = spx.tile([128, 512], BF16, tag="xt", name="xt")
                nc.sync.dma_start(xt[:], xT_d[k * 128 : (k + 1) * 128, 0:512])
                xts0.append(xt)
            nc.sync.dma_start(perm_sb[:], perm_d[:])
            nc.sync.dma_start(cos_sb[:], cos_d[:])
            nc.sync.dma_start(sin_sb[:], sin_d[:])
            nc.sync.dma_start(eye_sb[:], eye_d[:])
            nc.sync.dma_start(mask_sb[:], mask_d[:])
            nc.sync.dma_start(wf_sb[:], wf_d.rearrange("(c p) n -> p c n", p=128))
            nc.sync.dma_start(ones_sb[:], ones_d[:])
            # fp8e5 copy of the 0/1 mask (dtype-matched to the fp8 p tiles)
            nc.vector.tensor_copy(mask8_sb[:], mask_sb[:])
            for t in proj_tasks(0, xts0):
                t()

            # -- main loop -------------------------------------------------
            pending_norm = []  # deferred norm-finish closures

            def mk_norm(g, tci, o_ac):
                """Normalize o_ac rows 0:64 by the denominator in row 64 and
                store into oT_ab (bf16)."""
                tsl = slice(tci * 512, (tci + 1) * 512)
                odd = g % 2 == 1
                sums = spr.tile([128, 512], BF16, tag="rec", name="rec")
                nc.vector.tensor_copy(sums[64:65, :], o_ac[64:65, :])

                def run():
                    bc = ps.tile([128, 512], F32, tag="tmp", name="tmp")
                    nc.tensor.matmul(
                        bc[0:64, :], ones_sb[64:65, 0:64], sums[64:65, :],
                        start=True, stop=True,
                    )
                    bc_sb = spr.tile([64, 512], F32, tag="bcs", name="bcs")
                    nc.vector.reciprocal_approx_fast(out=bc_sb[:], in_=bc[0:64, :])
                    if odd:
                        stg = spr.tile([64, 512], BF16, tag="stg", name="stg")
                        nc.vector.tensor_mul(stg[:], o_ac[0:64, :], bc_sb[:])
                        nc.sync.dma_start(oT_ab[g // 2][64:128, tsl], stg[:])
                    else:
                        nc.vector.tensor_mul(
                            oT_ab[g // 2][0:64, tsl], o_ac[0:64, :], bc_sb[:]
                        )
                return run

            for tci in range(TCH):
                tsl0 = tci * 512
                njp = 2 * tci + 2

                bg = []  # (gate_seq | None, closure); seq = 2*chunk + pair
                if tci + 1 < TCH:
                    xts = load_x(tci + 1)
                    bg += [(None, t) for t in proj_tasks(tci + 1, xts)]
                if tci >= 1:
                    # final(tci-1) must not be emitted until every norm of
                    # chunk tci-1 has been emitted (it reads their oT writes)
                    bg += [(2 * tci - 1, t) for t in final_tasks(tci - 1)]
                if tci == TCH - 1:
                    # last chunk: run the pair-0 contraction half of its
                    # output projection during pair-1 attention
                    bg += [
                        (2 * tci, t)
                        for t in final_tasks(tci, ccs=(0,), dst=y2_d)
                    ]
                bg_done = 0
                bg_total = len(bg)
                slots = 2 * 2 * 2 * njp  # pairs x h01 x jp, x4 slot increment
                slot = 0

                def norms_pending_upto(c):
                    return any(ch <= c for ch, _ in pending_norm)

                for pair in range(2):
                    qc = qcat[pair]
                    o_acs = [
                        psacc.tile([128, 512], F32, tag="oacc", name="oacc")
                        for _ in range(2)
                    ]
                    DEPTH = 3
                    pvq = []  # (jp, h01, p_tile, win) awaiting PV emission

                    def emit_pv(jp, h01, p_t, win, o_acs=o_acs, njp=njp):
                        if jp == 0:
                            for i in range(2):
                                nc.tensor.matmul(
                                    o_acs[h01][0:65, :],
                                    v01_sb[:, i, 0:65],
                                    p_t[:, i, :],
                                    start=(i == 0),
                                    stop=False,
                                )
                        else:
                            nc.tensor.matmul(
                                o_acs[h01][0:65, win:512],
                                v8_sb[:, 2 * jp : 2 * jp + 2, 0:65],
                                p_t[:, :, win:512],
                                start=False,
                                stop=(jp == njp - 1),
                                perf_mode=DR,
                            )

                    for jp in range(njp):
                        win = 256 if jp == 2 * tci + 1 else 0
                        diag = jp in (2 * tci, 2 * tci + 1)
                        for h01 in range(2):
                            s_ps = pss.tile([128, 2, 512], F32, tag="s", name="s")
                            for i in range(2):
                                blk = 2 * jp + i
                                nc.tensor.matmul(
                                    s_ps[:, i, win:512],
                                    qkvT[2][0:64, blk * 128 : (blk + 1) * 128],
                                    qc[:, h01, tsl0 + win : tsl0 + 512],
                                    start=True,
                                    stop=True,
                                )
                            pdt = BF16 if jp == 0 else F8E5
                            p_t = spp.tile([128, 2, 512], pdt, tag="p", name="p")
                            nc.scalar.activation(
                                p_t[:, :, win:512], s_ps[:, :, win:512], AF.Exp
                            )
                            if diag:
                                nc.gpsimd.tensor_mul(
                                    p_t[:, :, win : win + 256],
                                    p_t[:, :, win : win + 256],
                                    mask_sb[:] if jp == 0 else mask8_sb[:],
                                )
                            pvq.append((jp, h01, p_t, win))
                            if len(pvq) > DEPTH:
                                emit_pv(*pvq.pop(0))
                        if pending_norm:
                            pending_norm.pop(0)[1]()
                        slot += 4
                        due = bg_total * min(slot, slots) // slots
                        while bg_done < due:
                            gate, fn = bg[bg_done]
                            if gate is not None and norms_pending_upto(gate):
                                break
                            fn()
                            bg_done += 1

                    for args in pvq:
                        emit_pv(*args)
                    for h01 in range(2):
                        g = pair * 2 + h01
                        pending_norm.append(
                            (2 * tci + pair, mk_norm(g, tci, o_acs[h01]))
                        )

                while bg_done < bg_total:
                    gate, fn = bg[bg_done]
                    if gate is not None and norms_pending_upto(gate):
                        # flush enough pending norms to unblock
                        while norms_pending_upto(gate):
                            pending_norm.pop(0)[1]()
                    fn()
                    bg_done += 1

            for _, fn in pending_norm:
                fn()
            for t in final_tasks(TCH - 1, ccs=(1,)):
                t()

    nc.compile()
    return nc


def host_shard(inputs):
    """Build the 8 per-core input maps from full inputs."""
    x = np.ascontiguousarray(np.asarray(inputs["input"], dtype=np.float32))
    W = np.asarray(inputs["W_attn"], dtype=np.float32)
    bb = np.asarray(inputs["b_attn"], dtype=np.float32)
    Wf = np.asarray(inputs["W_final"], dtype=np.float32)

    half = HS // 2
    inv_freq = MAX_PERIOD ** (-np.arange(half, dtype=np.float32) / half)
    ang = np.arange(T, dtype=np.float32)[:, None] * inv_freq  # (T, 32)
    sin_t = np.sin(ang).astype(np.float32)
    cos_t = np.cos(ang).astype(np.float32)
    cosT = np.repeat(cos_t.T, 2, axis=0)  # (64, T): row d -> cos(t*f[d//2])
    sgn = np.where(np.arange(HS) % 2 == 0, -1.0, 1.0).astype(np.float32)
    sinT = np.repeat(sin_t.T, 2, axis=0) * sgn[:, None]
    cos128 = np.ascontiguousarray(np.concatenate([cosT, cosT], axis=0))
    sin128 = np.ascontiguousarray(np.concatenate([sinT, sinT], axis=0))

    perm = np.zeros((128, 128), np.float32)
    idx = np.arange(128)
    perm[idx ^ 1, idx] = 1.0
    eye64 = np.zeros((128, 64), np.float32)
    eye64[64:128, :] = np.eye(64, dtype=np.float32)

    # 0/1 keep-mask for the two diagonal key-blocks of each chunk, applied on
    # the first 256 computed queries of a (block-pair, window):
    #   slot 0 (earlier block):  u<128: keep kk<=u ; u in [128,256): keep all
    #   slot 1 (later block):    u<128: keep none  ; u in [128,256): keep kk<=u-128
    kk = np.arange(128)[:, None]
    u = np.arange(256)[None, :]
    m0 = np.where(u < 128, (kk <= u), True)
    m1 = np.where(u < 128, False, (kk <= u - 128))
    mask01 = np.ascontiguousarray(
        np.stack([m0, m1], axis=1).astype(np.float32)
    )  # (128, 2, 256)

    ones64 = np.ones((128, 64), np.float32)

    in_maps = []
    for cid in range(8):
        b, h = cid // 4, cid % 4
        qcols = np.concatenate(
            [np.arange(g * 256 + h * 64, g * 256 + h * 64 + 64) for g in range(G)]
        )
        kcols = np.arange(1024 + h * 64, 1024 + h * 64 + 64)
        vcols = np.arange(1280 + h * 64, 1280 + h * 64 + 64)
        cols = np.concatenate([qcols, kcols, vcols])
        w_loc = W[:, cols].copy()
        b_loc = bb[cols].copy()
        w_loc[:, :256] *= SCALE
        b_loc[:256] *= SCALE
        b_loc_m = np.ascontiguousarray(b_loc.reshape(3, 128).T)  # (128, 3)

        rows = np.concatenate(
            [np.arange(g * 256 + h * 64, g * 256 + h * 64 + 64) for g in range(G)]
        )
        wf_loc = np.ascontiguousarray(Wf[rows, :])  # (256, 1024)

        in_maps.append(
            {
                "xT": np.ascontiguousarray(x[b].T).astype(ml_dtypes.bfloat16),
                "w_qkv": w_loc.astype(ml_dtypes.bfloat16),
                "b_loc": b_loc_m,
                "cosT": cos128.astype(ml_dtypes.bfloat16),
                "sinT": sin128.astype(ml_dtypes.bfloat16),
                "perm": perm.astype(ml_dtypes.bfloat16),
                "eye64": eye64.astype(ml_dtypes.bfloat16),
                "mask01": mask01.astype(ml_dtypes.bfloat16),
                "wf": wf_loc.astype(ml_dtypes.bfloat16),
                "onesd": ones64.astype(ml_dtypes.bfloat16),
            }
        )
    return in_maps


def host_unshard(results, b_final):
    """Sum the 4 per-h partial yT per batch (folding in the last chunk's
    early contraction-half partial y2T), add bias, transpose back."""
    out = np.empty((B, T, C), np.float32)
    for b in range(B):
        acc = results[b * 4]["yT"].astype(np.float32)
        acc[:, 3 * 512 :] += results[b * 4]["y2T"].astype(np.float32)
        for h in range(1, 4):
            acc = acc + results[b * 4 + h]["yT"]
            acc[:, 3 * 512 :] += results[b * 4 + h]["y2T"].astype(np.float32)
        out[b] = acc.T + b_final[None, :]
    return out


_NC_CACHE = None


def _get_nc():
    global _NC_CACHE
    if _NC_CACHE is None:
        _NC_CACHE = build_nc()
    return _NC_CACHE


def kernel(**inputs):
    nc = _get_nc()
    in_maps = host_shard(inputs)
    res = run_bass_kernel_spmd(nc, in_maps, core_ids=list(range(8)))
    return host_unshard(res.results, np.asarray(inputs["b_final"], dtype=np.float32))


# revision 37
# speedup vs baseline: 1.0840x; 1.0356x over previous
"""Trainium2 Bass kernel for grouped-query causal attention (B=2, T=2048, C=1024,
16 q heads / 4 kv heads, RoPE, fused qkv + output projection).

Sharding: 8 cores = (batch b, kv-head h). Each core:
  - projects x -> qT (4 heads), kT, vT with pre-sliced/pre-scaled bf16 weights
    (transposed layout: channels on partitions, T on free dim)
  - applies RoPE (pair-swap via permutation matmul on PE + DVE mul/add)
  - causal attention for its 4 query heads: S^T blocks in bf16, exp without
    max-subtraction (logits are O(7)), post-exp 0/1 causal mask on gpsimd,
    softmax denominators via a ones column appended to V
  - PV: first key-block-pair (keys 0-255) in bf16 (protects small-window
    queries), remaining block-pairs via fp8 DoubleRow matmuls
    (p in e5m2, V in e4m3) -- one matmul per TWO key blocks at 0.5 cyc/row
  - partial output projection y^T = Wf_local^T @ oT (bf16); final bias is
    added on the host after summing the 4 per-h partials per batch.
"""

import sys

sys.path.insert(0, "/opt/trn_rl_repo")

import ml_dtypes
import numpy as np

import concourse.bacc as bacc
import concourse.mybir as mybir
from concourse import tile
from concourse.bass_utils import run_bass_kernel_spmd

B, T, C = 2, 2048, 1024
G, HKV, HS = 4, 4, 64
OUT_DIM = C + 2 * (C // G)
SCALE = 1.0 / np.sqrt(HS)
MAX_PERIOD = 10000.0

F32 = mybir.dt.float32
BF16 = mybir.dt.bfloat16
F8E4 = mybir.dt.float8e4
F8E5 = mybir.dt.float8e5
AF = mybir.ActivationFunctionType
DR = mybir.MatmulPerfMode.DoubleRow

TCH = T // 512  # 4 chunks of 512 along T
NT = T // 128  # 16 tiles of 128 along T


def build_nc():
    nc = bacc.Bacc(None, target_bir_lowering=False)

    xT_d = nc.dram_tensor("xT", [C, T], BF16, kind="ExternalInput")
    w_d = nc.dram_tensor("w_qkv", [C, 384], BF16, kind="ExternalInput")
    bl_d = nc.dram_tensor("b_loc", [128, 3], F32, kind="ExternalInput")
    cos_d = nc.dram_tensor("cosT", [128, T], BF16, kind="ExternalInput")
    sin_d = nc.dram_tensor("sinT", [128, T], BF16, kind="ExternalInput")
    perm_d = nc.dram_tensor("perm", [128, 128], BF16, kind="ExternalInput")
    eye_d = nc.dram_tensor("eye64", [128, 64], BF16, kind="ExternalInput")
    mask_d = nc.dram_tensor("mask01", [128, 2, 256], BF16, kind="ExternalInput")
    wf_d = nc.dram_tensor("wf", [256, 1024], BF16, kind="ExternalInput")
    ones_d = nc.dram_tensor("onesd", [128, 64], BF16, kind="ExternalInput")
    yT_d = nc.dram_tensor("yT", [C, T], BF16, kind="ExternalOutput")
    # first-contraction-half partial of the last chunk's output projection
    # (runs early, during pair-1 attention; host adds it back in)
    y2_d = nc.dram_tensor("y2T", [C, 512], BF16, kind="ExternalOutput")

    with tile.TileContext(nc) as tc:
        with (
            tc.tile_pool(name="persist", bufs=1) as pp,
            tc.tile_pool(name="xstream", bufs=10) as spx,
            tc.tile_pool(name="pstream", bufs=6) as spp,
            tc.tile_pool(name="rstream", bufs=3) as spr,
            tc.tile_pool(name="ostream", bufs=3) as spo,
            tc.tile_pool(name="ps_acc", bufs=3, space="PSUM") as psacc,
            tc.tile_pool(name="ps_s", bufs=2, space="PSUM") as pss,
            tc.tile_pool(name="ps_tmp", bufs=1, space="PSUM") as ps,
        ):
            # ---- persistent tiles ----
            w_sb = pp.tile([128, 8, 384], BF16, tag="w", name="w")
            bl_sb = pp.tile([128, 3], F32, tag="bl", name="bl")
            cos_sb = pp.tile([128, T], BF16, tag="cos", name="cos")
            sin_sb = pp.tile([128, T], BF16, tag="sin", name="sin")
            perm_sb = pp.tile([128, 128], BF16, tag="perm", name="perm")
            eye_sb = pp.tile([128, 64], BF16, tag="eye", name="eye")
            mask_sb = pp.tile([128, 2, 256], BF16, tag="mask", name="mask")
            mask8_sb = pp.tile([128, 2, 256], F8E5, tag="mask8", name="mask8")
            wf_sb = pp.tile([128, 2, 1024], BF16, tag="wf", name="wf")
            ones_sb = pp.tile([128, 64], BF16, tag="ones", name="ones")
            qkvT = [pp.tile([128, T], BF16, tag=f"qkvT{m}", name=f"qkvT{m}") for m in range(3)]
            qcat = [pp.tile([64, 2, T], BF16, tag=f"qcat{m}", name=f"qcat{m}") for m in range(2)]
            v8_sb = pp.tile([128, NT, 80], F8E4, tag="v8", name="v8")
            v01_sb = pp.tile([128, 2, 65], BF16, tag="v01", name="v01")
            oT_ab = [pp.tile([128, T], BF16, tag=f"oT{i}", name=f"oT{i}") for i in range(2)]

            nc.sync.dma_start(bl_sb[:], bl_d[:])
            nc.gpsimd.memset(v8_sb[:, :, 64:65], 1.0)
            nc.gpsimd.memset(v01_sb[:, :, 64:65], 1.0)

            # -- emission helpers ------------------------------------------
            def load_x(tc_i):
                tsl = slice(tc_i * 512, (tc_i + 1) * 512)
                xts = []
                for k in range(8):
                    xt = spx.tile([128, 512], BF16, tag="xt", name="xt")
                    nc.sync.dma_start(xt[:], xT_d[k * 128 : (k + 1) * 128, tsl])
                    xts.append(xt)
                return xts

            def proj_tasks(tc_i, xts):
                """Dense background tasks for chunk tc_i's projection+RoPE+vT.
                Ordered so k/v are ready before q heads (attention needs k,v
                first at chunk 0)."""
                tsl = slice(tc_i * 512, (tc_i + 1) * 512)

                def mk_group(mt):
                    def run():
                        pr = ps.tile([128, 512], F32, tag="tmp", name="tmp")
                        for k in range(8):
                            nc.tensor.matmul(
                                pr[:],
                                w_sb[:, k, mt * 128 : (mt + 1) * 128],
                                xts[k][:],
                                start=(k == 0),
                                stop=(k == 7),
                            )
                        nc.vector.tensor_scalar_add(
                            qkvT[mt][:, tsl], pr[:], bl_sb[:, mt : mt + 1]
                        )
                    return run

                def mk_rope_q(mt):
                    def run():
                        # pair-swap via strided SBUF->SBUF DMA (even<->odd
                        # partitions) instead of a PE permutation matmul
                        sw = spp.tile([128, 2, 512], BF16, tag="p", name="p")
                        qv = qkvT[mt][:, tsl].rearrange("(j two) t -> j two t", two=2)
                        swv = sw[:, 0, :].rearrange("(j two) t -> j two t", two=2)
                        nc.sync.dma_start(swv[:, 0, :], qv[:, 1, :])
                        nc.sync.dma_start(swv[:, 1, :], qv[:, 0, :])
                        nc.vector.tensor_mul(sw[:, 1, :], sw[:, 0, :], sin_sb[:, tsl])
                        nc.vector.tensor_mul(
                            qkvT[mt][:, tsl], qkvT[mt][:, tsl], cos_sb[:, tsl]
                        )
                        nc.vector.tensor_add(
                            qkvT[mt][:, tsl], qkvT[mt][:, tsl], sw[:, 1, :]
                        )
                        nc.sync.dma_start(qcat[mt][:, 0, tsl], qkvT[mt][0:64, tsl])
                        nc.sync.dma_start(qcat[mt][:, 1, tsl], qkvT[mt][64:128, tsl])
                    return run

                def mk_vt(i):
                    def run():
                        tt = tc_i * 4 + i
                        vt = ps.tile([128, 512], BF16, tag="tmp", name="tmp")
                        nc.tensor.transpose(
                            vt[:, 0:64],
                            qkvT[2][64:128, tt * 128 : (tt + 1) * 128],
                            eye_sb[64:128, :],
                        )
                        nc.vector.tensor_copy(v8_sb[:, tt, 0:64], vt[:, 0:64])
                        if tt < 2:
                            nc.vector.tensor_copy(v01_sb[:, tt, 0:64], vt[:, 0:64])
                    return run

                def rope_k():
                    sw = spp.tile([128, 2, 512], BF16, tag="p", name="p")
                    kv = qkvT[2][0:64, tsl].rearrange("(j two) t -> j two t", two=2)
                    swv = sw[0:64, 0, :].rearrange("(j two) t -> j two t", two=2)
                    nc.sync.dma_start(swv[:, 0, :], kv[:, 1, :])
                    nc.sync.dma_start(swv[:, 1, :], kv[:, 0, :])
                    nc.vector.tensor_mul(
                        sw[0:64, 1, :], sw[0:64, 0, :], sin_sb[0:64, tsl]
                    )
                    nc.vector.tensor_mul(
                        qkvT[2][0:64, tsl], qkvT[2][0:64, tsl], cos_sb[0:64, tsl]
                    )
                    nc.vector.tensor_add(
                        qkvT[2][0:64, tsl], qkvT[2][0:64, tsl], sw[0:64, 1, :]
                    )

                return [
                    mk_group(2), rope_k,
                    mk_vt(0), mk_vt(1), mk_vt(2), mk_vt(3),
                    mk_group(0), mk_rope_q(0),
                    mk_group(1), mk_rope_q(1),
                ]

            def final_tasks(tc_i, ccs=(0, 1), dst=None):
                tsl = slice(tc_i * 512, (tc_i + 1) * 512)
                d = yT_d if dst is None else dst

                def mk(nt):
                    def run():
                        y_ps = ps.tile([128, 512], F32, tag="tmp", name="tmp")
                        for j, cc in enumerate(ccs):
                            nc.tensor.matmul(
                                y_ps[:],
                                wf_sb[:, cc, nt * 128 : (nt + 1) * 128],
                                oT_ab[cc][:, tsl],
                                start=(j == 0),
                                stop=(j == len(ccs) - 1),
                            )
                        y_sb = spo.tile([128, 512], BF16, tag="yout", name="yout")
                        nc.vector.tensor_copy(y_sb[:], y_ps[:])
                        if dst is None:
                            nc.sync.dma_start(d[nt * 128 : (nt + 1) * 128, tsl], y_sb[:])
                        else:
                            nc.sync.dma_start(d[nt * 128 : (nt + 1) * 128, :], y_sb[:])
                    return run

                return [mk(nt) for nt in range(8)]

            # -- prologue --------------------------------------------------
            # interleave per-k weight and x-tile loads so proj matmul k can
            # start as soon as its own (w_k, x_k) pair has landed
            xts0 = []
            for k in range(8):
                nc.sync.dma_start(w_sb[:, k, :], w_d[k * 128 : (k + 1) * 128, :])
                xt = spx.tile([128, 512], BF16, tag="xt", name="xt")
                nc.sync.dma_start(xt[:], xT_d[k * 128 : (k + 1) * 128, 0:512])
                xts0.append(xt)
            nc.sync.dma_start(perm_sb[:], perm_d[:])
            nc.sync.dma_start(cos_sb[:], cos_d[:])
            nc.sync.dma_start(sin_sb[:], sin_d[:])
            nc.sync.dma_start(eye_sb[:], eye_d[:])
            nc.sync.dma_start(mask_sb[:], mask_d[:])
            nc.sync.dma_start(wf_sb[:], wf_d.rearrange("(c p) n -> p c n", p=128))
            nc.sync.dma_start(ones_sb[:], ones_d[:])
            # fp8e5 copy of the 0/1 mask (dtype-matched to the fp8 p tiles)
            nc.vector.tensor_copy(mask8_sb[:], mask_sb[:])
            for t in proj_tasks(0, xts0):
                t()

            # -- main loop -------------------------------------------------
            pending_norm = []  # deferred norm-finish closures

            def mk_norm(g, tci, o_ac):
                """Normalize o_ac rows 0:64 by the denominator in row 64 and
                store into oT_ab (bf16)."""
                tsl = slice(tci * 512, (tci + 1) * 512)
                odd = g % 2 == 1
                sums = spr.tile([128, 512], BF16, tag="rec", name="rec")
                nc.vector.tensor_copy(sums[64:65, :], o_ac[64:65, :])

                def run():
                    bc = ps.tile([128, 512], F32, tag="tmp", name="tmp")
                    nc.tensor.matmul(
                        bc[0:64, :], ones_sb[64:65, 0:64], sums[64:65, :],
                        start=True, stop=True,
                    )
                    bc_sb = spr.tile([64, 512], F32, tag="bcs", name="bcs")
                    nc.vector.reciprocal_approx_fast(out=bc_sb[:], in_=bc[0:64, :])
                    if odd:
                        stg = spr.tile([64, 512], BF16, tag="stg", name="stg")
                        nc.vector.tensor_mul(stg[:], o_ac[0:64, :], bc_sb[:])
                        nc.sync.dma_start(oT_ab[g // 2][64:128, tsl], stg[:])
                    else:
                        nc.vector.tensor_mul(
                            oT_ab[g // 2][0:64, tsl], o_ac[0:64, :], bc_sb[:]
                        )
                return run

            for tci in range(TCH):
                tsl0 = tci * 512
                njp = 2 * tci + 2

                bg = []  # (gate_seq | None, closure); seq = 2*chunk + pair
                if tci + 1 < TCH:
                    xts = load_x(tci + 1)
                    bg += [(None, t) for t in proj_tasks(tci + 1, xts)]
                if tci >= 1:
                    # final(tci-1) must not be emitted until every norm of
                    # chunk tci-1 has been emitted (it reads their oT writes)
                    bg += [(2 * tci - 1, t) for t in final_tasks(tci - 1)]
                if tci == TCH - 1:
                    # last chunk: run the pair-0 contraction half of its
                    # output projection during pair-1 attention
                    bg += [
                        (2 * tci, t)
                        for t in final_tasks(tci, ccs=(0,), dst=y2_d)
                    ]
                bg_done = 0
                bg_total = len(bg)
                slots = 2 * 2 * 2 * njp  # pairs x h01 x jp, x4 slot increment
                slot = 0

                def norms_pending_upto(c):
                    return any(ch <= c for ch, _ in pending_norm)

                for pair in range(2):
                    qc = qcat[pair]
                    o_acs = [
                        psacc.tile([128, 512], F32, tag="oacc", name="oacc")
                        for _ in range(2)
                    ]
                    DEPTH = 3
                    pvq = []  # (jp, h01, p_tile, win) awaiting PV emission

                    def emit_pv(jp, h01, p_t, win, o_acs=o_acs, njp=njp):
                        if jp == 0:
                            for i in range(2):
                                nc.tensor.matmul(
                                    o_acs[h01][0:65, :],
                                    v01_sb[:, i, 0:65],
                                    p_t[:, i, :],
                                    start=(i == 0),
                                    stop=False,
                                )
                        else:
                            nc.tensor.matmul(
                                o_acs[h01][0:65, win:512],
                                v8_sb[:, 2 * jp : 2 * jp + 2, 0:65],
                                p_t[:, :, win:512],
                                start=False,
                                stop=(jp == njp - 1),
                                perf_mode=DR,
                            )

                    for jp in range(njp):
                        win = 256 if jp == 2 * tci + 1 else 0
                        diag = jp in (2 * tci, 2 * tci + 1)
                        for h01 in range(2):
                            s_ps = pss.tile([128, 2, 512], F32, tag="s", name="s")
                            for i in range(2):
                                blk = 2 * jp + i
                                nc.tensor.matmul(
                                    s_ps[:, i, win:512],
                                    qkvT[2][0:64, blk * 128 : (blk + 1) * 128],
                                    qc[:, h01, tsl0 + win : tsl0 + 512],
                                    start=True,
                                    stop=True,
                                )
                            pdt = BF16 if jp == 0 else F8E5
                            p_t = spp.tile([128, 2, 512], pdt, tag="p", name="p")
                            nc.scalar.activation(
                                p_t[:, :, win:512], s_ps[:, :, win:512], AF.Exp
                            )
                            if diag:
                                nc.gpsimd.tensor_mul(
                                    p_t[:, :, win : win + 256],
                                    p_t[:, :, win : win + 256],
                                    mask_sb[:] if jp == 0 else mask8_sb[:],
                                )
                            pvq.append((jp, h01, p_t, win))
                            if len(pvq) > DEPTH:
                                emit_pv(*pvq.pop(0))
                        if pending_norm:
                            pending_norm.pop(0)[1]()
                        slot += 4
                        due = bg_total * min(slot, slots) // slots
                        while bg_done < due:
                            gate, fn = bg[bg_done]
                            if gate is not None and norms_pending_upto(gate):
                                break
                            fn()
                            bg_done += 1

                    for args in pvq:
                        emit_pv(*args)
                    for h01 in range(2):
                        g = pair * 2 + h01
                        pending_norm.append(
                            (2 * tci + pair, mk_norm(g, tci, o_acs[h01]))
                        )

                while bg_done < bg_total:
                    gate, fn = bg[bg_done]
                    if gate is not None and norms_pending_upto(gate):
                        # flush enough pending norms to unblock
                        while norms_pending_upto(gate):
                            pending_norm.pop(0)[1]()
                    fn()
                    bg_done += 1

            for _, fn in pending_norm:
                fn()
            for t in final_tasks(TCH - 1, ccs=(1,)):
                t()

    nc.compile()
    return nc


def host_shard(inputs):
    """Build the 8 per-core input maps from full inputs."""
    x = np.ascontiguousarray(np.asarray(inputs["input"], dtype=np.float32))
    W = np.asarray(inputs["W_attn"], dtype=np.float32)
    bb = np.asarray(inputs["b_attn"], dtype=np.float32)
    Wf = np.asarray(inputs["W_final"], dtype=np.float32)

    half = HS // 2
    inv_freq = MAX_PERIOD ** (-np.arange(half, dtype=np.float32) / half)
    ang = np.arange(T, dtype=np.float32)[:, None] * inv_freq  # (T, 32)
    sin_t = np.sin(ang).astype(np.float32)
    cos_t = np.cos(ang).astype(np.float32)
    cosT = np.repeat(cos_t.T, 2, axis=0)  # (64, T): row d -> cos(t*f[d//2])
    sgn = np.where(np.arange(HS) % 2 == 0, -1.0, 1.0).astype(np.float32)
    sinT = np.repeat(sin_t.T, 2, axis=0) * sgn[:, None]
    cos128 = np.ascontiguousarray(np.concatenate([cosT, cosT], axis=0))
    sin128 = np.ascontiguousarray(np.concatenate([sinT, sinT], axis=0))

    perm = np.zeros((128, 128), np.float32)
    idx = np.arange(128)
    perm[idx ^ 1, idx] = 1.0
    eye64 = np.zeros((128, 64), np.float32)
    eye64[64:128, :] = np.eye(64, dtype=np.float32)

    # 0/1 keep-mask for the two diagonal key-blocks of each chunk, applied on
    # the first 256 computed queries of a (block-pair, window):
    #   slot 0 (earlier block):  u<128: keep kk<=u ; u in [128,256): keep all
    #   slot 1 (later block):    u<128: keep none  ; u in [128,256): keep kk<=u-128
    kk = np.arange(128)[:, None]
    u = np.arange(256)[None, :]
    m0 = np.where(u < 128, (kk <= u), True)
    m1 = np.where(u < 128, False, (kk <= u - 128))
    mask01 = np.ascontiguousarray(
        np.stack([m0, m1], axis=1).astype(np.float32)
    )  # (128, 2, 256)

    ones64 = np.ones((128, 64), np.float32)

    in_maps = []
    for cid in range(8):
        b, h = cid // 4, cid % 4
        qcols = np.concatenate(
            [np.arange(g * 256 + h * 64, g * 256 + h * 64 + 64) for g in range(G)]
        )
        kcols = np.arange(1024 + h * 64, 1024 + h * 64 + 64)
        vcols = np.arange(1280 + h * 64, 1280 + h * 64 + 64)
        cols = np.concatenate([qcols, kcols, vcols])
        w_loc = W[:, cols].copy()
        b_loc = bb[cols].copy()
        w_loc[:, :256] *= SCALE
        b_loc[:256] *= SCALE
        b_loc_m = np.ascontiguousarray(b_loc.reshape(3, 128).T)  # (128, 3)

        rows = np.concatenate(
            [np.arange(g * 256 + h * 64, g * 256 + h * 64 + 64) for g in range(G)]
        )
        wf_loc = np.ascontiguousarray(Wf[rows, :])  # (256, 1024)

        in_maps.append(
            {
                "xT": np.ascontiguousarray(x[b].T).astype(ml_dtypes.bfloat16),
                "w_qkv": w_loc.astype(ml_dtypes.bfloat16),
                "b_loc": b_loc_m,
                "cosT": cos128.astype(ml_dtypes.bfloat16),
                "sinT": sin128.astype(ml_dtypes.bfloat16),
                "perm": perm.astype(ml_dtypes.bfloat16),
                "eye64": eye64.astype(ml_dtypes.bfloat16),
                "mask01": mask01.astype(ml_dtypes.bfloat16),
                "wf": wf_loc.astype(ml_dtypes.bfloat16),
                "onesd": ones64.astype(ml_dtypes.bfloat16),
            }
        )
    return in_maps


def host_unshard(results, b_final):
    """Sum the 4 per-h partial yT per batch (folding in the last chunk's
    early contraction-half partial y2T), add bias, transpose back."""
    out = np.empty((B, T, C), np.float32)
    for b in range(B):
        acc = results[b * 4]["yT"].astype(np.float32)
        acc[:, 3 * 512 :] += results[b * 4]["y2T"].astype(np.float32)
        for h in range(1, 4):
            acc = acc + results[b * 4 + h]["yT"]
            acc[:, 3 * 512 :] += results[b * 4 + h]["y2T"].astype(np.float32)
        out[b] = acc.T + b_final[None, :]
    return out


_NC_CACHE = None


def _get_nc():
    global _NC_CACHE
    if _NC_CACHE is None:
        _NC_CACHE = build_nc()
    return _NC_CACHE


def kernel(**inputs):
    nc = _get_nc()
    in_maps = host_shard(inputs)
    res = run_bass_kernel_spmd(nc, in_maps, core_ids=list(range(8)))
    return host_unshard(res.results, np.asarray(inputs["b_final"], dtype=np.float32))


# revision 42
# speedup vs baseline: 1.0966x; 1.0116x over previous
"""Trainium2 Bass kernel for grouped-query causal attention (B=2, T=2048, C=1024,
16 q heads / 4 kv heads, RoPE, fused qkv + output projection).

Sharding: 8 cores = (batch b, kv-head h). Each core:
  - projects x -> qT (4 heads), kT, vT with pre-sliced/pre-scaled bf16 weights
    (transposed layout: channels on partitions, T on free dim)
  - applies RoPE (pair-swap via permutation matmul on PE + DVE mul/add)
  - causal attention for its 4 query heads: S^T blocks in bf16, exp without
    max-subtraction (logits are O(7)), post-exp 0/1 causal mask on gpsimd,
    softmax denominators via a ones column appended to V
  - PV: first key-block-pair (keys 0-255) in bf16 (protects small-window
    queries), remaining block-pairs via fp8 DoubleRow matmuls
    (p in e5m2, V in e4m3) -- one matmul per TWO key blocks at 0.5 cyc/row
  - partial output projection y^T = Wf_local^T @ oT (bf16); final bias is
    added on the host after summing the 4 per-h partials per batch.
"""

import sys

sys.path.insert(0, "/opt/trn_rl_repo")

import ml_dtypes
import numpy as np

import concourse.bacc as bacc
import concourse.mybir as mybir
from concourse import tile
from concourse.bass_utils import run_bass_kernel_spmd

B, T, C = 2, 2048, 1024
G, HKV, HS = 4, 4, 64
OUT_DIM = C + 2 * (C // G)
SCALE = 1.0 / np.sqrt(HS)
MAX_PERIOD = 10000.0

F32 = mybir.dt.float32
BF16 = mybir.dt.bfloat16
F8E4 = mybir.dt.float8e4
F8E5 = mybir.dt.float8e5
AF = mybir.ActivationFunctionType
DR = mybir.MatmulPerfMode.DoubleRow

TCH = T // 512  # 4 chunks of 512 along T
NT = T // 128  # 16 tiles of 128 along T


def build_nc():
    nc = bacc.Bacc(None, target_bir_lowering=False)

    xT_d = nc.dram_tensor("xT", [C, T], BF16, kind="ExternalInput")
    w_d = nc.dram_tensor("w_qkv", [C, 384], BF16, kind="ExternalInput")
    bl_d = nc.dram_tensor("b_loc", [128, 3], F32, kind="ExternalInput")
    cos_d = nc.dram_tensor("cosT", [128, T], BF16, kind="ExternalInput")
    sin_d = nc.dram_tensor("sinT", [128, T], BF16, kind="ExternalInput")
    perm_d = nc.dram_tensor("perm", [128, 128], BF16, kind="ExternalInput")
    eye_d = nc.dram_tensor("eye64", [128, 64], BF16, kind="ExternalInput")
    mask_d = nc.dram_tensor("mask01", [128, 2, 256], BF16, kind="ExternalInput")
    wf_d = nc.dram_tensor("wf", [256, 1024], BF16, kind="ExternalInput")
    ones_d = nc.dram_tensor("onesd", [128, 64], BF16, kind="ExternalInput")
    yT_d = nc.dram_tensor("yT", [C, T], BF16, kind="ExternalOutput")
    # first-contraction-half partial of the last chunk's output projection
    # (runs early, during pair-1 attention; host adds it back in)
    y2_d = nc.dram_tensor("y2T", [C, 512], BF16, kind="ExternalOutput")

    with tile.TileContext(nc) as tc:
        with (
            tc.tile_pool(name="persist", bufs=1) as pp,
            tc.tile_pool(name="xstream", bufs=10) as spx,
            tc.tile_pool(name="pstream", bufs=6) as spp,
            tc.tile_pool(name="rstream", bufs=3) as spr,
            tc.tile_pool(name="ostream", bufs=3) as spo,
            tc.tile_pool(name="ps_acc", bufs=3, space="PSUM") as psacc,
            tc.tile_pool(name="ps_s", bufs=2, space="PSUM") as pss,
            tc.tile_pool(name="ps_tmp", bufs=1, space="PSUM") as ps,
        ):
            # ---- persistent tiles ----
            w_sb = pp.tile([128, 8, 384], BF16, tag="w", name="w")
            bl_sb = pp.tile([128, 3], F32, tag="bl", name="bl")
            cos_sb = pp.tile([128, T], BF16, tag="cos", name="cos")
            sin_sb = pp.tile([128, T], BF16, tag="sin", name="sin")
            perm_sb = pp.tile([128, 128], BF16, tag="perm", name="perm")
            eye_sb = pp.tile([128, 64], BF16, tag="eye", name="eye")
            mask_sb = pp.tile([128, 2, 256], BF16, tag="mask", name="mask")
            mask8_sb = pp.tile([128, 2, 256], F8E5, tag="mask8", name="mask8")
            wf_sb = pp.tile([128, 2, 1024], BF16, tag="wf", name="wf")
            ones_sb = pp.tile([128, 64], BF16, tag="ones", name="ones")
            qkvT = [pp.tile([128, T], BF16, tag=f"qkvT{m}", name=f"qkvT{m}") for m in range(3)]
            qcat = [pp.tile([64, 2, T], BF16, tag=f"qcat{m}", name=f"qcat{m}") for m in range(2)]
            v8_sb = pp.tile([128, NT, 80], F8E4, tag="v8", name="v8")
            v01_sb = pp.tile([128, 2, 65], BF16, tag="v01", name="v01")
            oT_ab = [pp.tile([128, T], BF16, tag=f"oT{i}", name=f"oT{i}") for i in range(2)]

            nc.sync.dma_start(bl_sb[:], bl_d[:])
            nc.gpsimd.memset(v8_sb[:, :, 64:65], 1.0)
            nc.gpsimd.memset(v01_sb[:, :, 64:65], 1.0)

            # -- emission helpers ------------------------------------------
            def load_x(tc_i):
                """One merged DMA per chunk (1KB-contiguous rows): a single
                sync-queue slot instead of eight."""
                tsl = slice(tc_i * 512, (tc_i + 1) * 512)
                xt8 = spx.tile([128, 8, 512], BF16, tag="xt8", bufs=2, name="xt8")
                nc.sync.dma_start(
                    xt8[:], xT_d[:, tsl].rearrange("(k p) t -> p k t", p=128)
                )
                return lambda k: xt8[:, k, :]

            def proj_tasks(tc_i, xs):
                """Dense background tasks for chunk tc_i's projection+RoPE+vT.
                `xs(k)` yields the k-th 128-channel x tile AP. Ordered so k/v
                are ready before q heads (attention needs k,v first at chunk
                0)."""
                tsl = slice(tc_i * 512, (tc_i + 1) * 512)

                def mk_group(mt):
                    def run():
                        pr = ps.tile([128, 512], F32, tag="tmp", name="tmp")
                        for k in range(8):
                            nc.tensor.matmul(
                                pr[:],
                                w_sb[:, k, mt * 128 : (mt + 1) * 128],
                                xs(k),
                                start=(k == 0),
                                stop=(k == 7),
                            )
                        nc.vector.tensor_scalar_add(
                            qkvT[mt][:, tsl], pr[:], bl_sb[:, mt : mt + 1]
                        )
                    return run

                def mk_rope_q(mt):
                    def run():
                        # pair-swap via strided SBUF->SBUF DMA (even<->odd
                        # partitions) instead of a PE permutation matmul
                        sw = spp.tile([128, 2, 512], BF16, tag="p", name="p")
                        qv = qkvT[mt][:, tsl].rearrange("(j two) t -> j two t", two=2)
                        swv = sw[:, 0, :].rearrange("(j two) t -> j two t", two=2)
                        nc.sync.dma_start(swv[:, 0, :], qv[:, 1, :])
                        nc.sync.dma_start(swv[:, 1, :], qv[:, 0, :])
                        nc.vector.tensor_mul(sw[:, 1, :], sw[:, 0, :], sin_sb[:, tsl])
                        nc.vector.tensor_mul(
                            qkvT[mt][:, tsl], qkvT[mt][:, tsl], cos_sb[:, tsl]
                        )
                        nc.vector.tensor_add(
                            qkvT[mt][:, tsl], qkvT[mt][:, tsl], sw[:, 1, :]
                        )
                        nc.sync.dma_start(qcat[mt][:, 0, tsl], qkvT[mt][0:64, tsl])
                        nc.sync.dma_start(qcat[mt][:, 1, tsl], qkvT[mt][64:128, tsl])
                    return run

                def mk_vt(i):
                    def run():
                        tt = tc_i * 4 + i
                        vt = ps.tile([128, 512], BF16, tag="tmp", name="tmp")
                        nc.tensor.transpose(
                            vt[:, 0:64],
                            qkvT[2][64:128, tt * 128 : (tt + 1) * 128],
                            eye_sb[64:128, :],
                        )
                        nc.vector.tensor_copy(v8_sb[:, tt, 0:64], vt[:, 0:64])
                        if tt < 2:
                            nc.vector.tensor_copy(v01_sb[:, tt, 0:64], vt[:, 0:64])
                    return run

                def rope_k():
                    sw = spp.tile([128, 2, 512], BF16, tag="p", name="p")
                    kv = qkvT[2][0:64, tsl].rearrange("(j two) t -> j two t", two=2)
                    swv = sw[0:64, 0, :].rearrange("(j two) t -> j two t", two=2)
                    nc.sync.dma_start(swv[:, 0, :], kv[:, 1, :])
                    nc.sync.dma_start(swv[:, 1, :], kv[:, 0, :])
                    nc.vector.tensor_mul(
                        sw[0:64, 1, :], sw[0:64, 0, :], sin_sb[0:64, tsl]
                    )
                    nc.vector.tensor_mul(
                        qkvT[2][0:64, tsl], qkvT[2][0:64, tsl], cos_sb[0:64, tsl]
                    )
                    nc.vector.tensor_add(
                        qkvT[2][0:64, tsl], qkvT[2][0:64, tsl], sw[0:64, 1, :]
                    )

                return [
                    mk_group(2), rope_k,
                    mk_vt(0), mk_vt(1), mk_vt(2), mk_vt(3),
                    mk_group(0), mk_rope_q(0),
                    mk_group(1), mk_rope_q(1),
                ]

            def final_tasks(tc_i, ccs=(0, 1), dst=None):
                tsl = slice(tc_i * 512, (tc_i + 1) * 512)
                d = yT_d if dst is None else dst

                def mk(nt):
                    def run():
                        y_ps = ps.tile([128, 512], F32, tag="tmp", name="tmp")
                        for j, cc in enumerate(ccs):
                            nc.tensor.matmul(
                                y_ps[:],
                                wf_sb[:, cc, nt * 128 : (nt + 1) * 128],
                                oT_ab[cc][:, tsl],
                                start=(j == 0),
                                stop=(j == len(ccs) - 1),
                            )
                        y_sb = spo.tile([128, 512], BF16, tag="yout", name="yout")
                        nc.vector.tensor_copy(y_sb[:], y_ps[:])
                        if dst is None:
                            nc.sync.dma_start(d[nt * 128 : (nt + 1) * 128, tsl], y_sb[:])
                        else:
                            nc.sync.dma_start(d[nt * 128 : (nt + 1) * 128, :], y_sb[:])
                    return run

                return [mk(nt) for nt in range(8)]

            # -- prologue --------------------------------------------------
            # interleave per-k weight and x-tile loads so proj matmul k can
            # start as soon as its own (w_k, x_k) pair has landed
            xts0 = []
            for k in range(8):
                nc.sync.dma_start(w_sb[:, k, :], w_d[k * 128 : (k + 1) * 128, :])
                xt = spx.tile([128, 512], BF16, tag="xt", name="xt")
                nc.sync.dma_start(xt[:], xT_d[k * 128 : (k + 1) * 128, 0:512])
                xts0.append(xt)
            nc.sync.dma_start(perm_sb[:], perm_d[:])
            xs0 = lambda k: xts0[k][:]
            nc.sync.dma_start(cos_sb[:], cos_d[:])
            nc.sync.dma_start(sin_sb[:], sin_d[:])
            nc.sync.dma_start(eye_sb[:], eye_d[:])
            nc.sync.dma_start(mask_sb[:], mask_d[:])
            nc.sync.dma_start(wf_sb[:], wf_d.rearrange("(c p) n -> p c n", p=128))
            nc.sync.dma_start(ones_sb[:], ones_d[:])
            # fp8e5 copy of the 0/1 mask (dtype-matched to the fp8 p tiles)
            nc.vector.tensor_copy(mask8_sb[:], mask_sb[:])
            for t in proj_tasks(0, xs0):
                t()

            # -- main loop -------------------------------------------------
            pending_norm = []  # deferred norm-finish closures

            def mk_norm(g, tci, o_ac):
                """Normalize o_ac rows 0:64 by the denominator in row 64 and
                store into oT_ab (bf16)."""
                tsl = slice(tci * 512, (tci + 1) * 512)
                odd = g % 2 == 1
                sums = spr.tile([128, 512], BF16, tag="rec", name="rec")
                nc.vector.tensor_copy(sums[64:65, :], o_ac[64:65, :])

                def run():
                    bc = ps.tile([128, 512], F32, tag="tmp", name="tmp")
                    nc.tensor.matmul(
                        bc[0:64, :], ones_sb[64:65, 0:64], sums[64:65, :],
                        start=True, stop=True,
                    )
                    bc_sb = spr.tile([64, 512], F32, tag="bcs", name="bcs")
                    nc.vector.reciprocal_approx_fast(out=bc_sb[:], in_=bc[0:64, :])
                    if odd:
                        stg = spr.tile([64, 512], BF16, tag="stg", name="stg")
                        nc.vector.tensor_mul(stg[:], o_ac[0:64, :], bc_sb[:])
                        nc.sync.dma_start(oT_ab[g // 2][64:128, tsl], stg[:])
                    else:
                        nc.vector.tensor_mul(
                            oT_ab[g // 2][0:64, tsl], o_ac[0:64, :], bc_sb[:]
                        )
                return run

            for tci in range(TCH):
                tsl0 = tci * 512
                njp = 2 * tci + 2

                bg = []  # (gate_seq | None, closure); seq = 2*chunk + pair
                if tci + 1 < TCH:
                    xs = load_x(tci + 1)
                    bg += [(None, t) for t in proj_tasks(tci + 1, xs)]
                if tci >= 1:
                    # final(tci-1) must not be emitted until every norm of
                    # chunk tci-1 has been emitted (it reads their oT writes)
                    bg += [(2 * tci - 1, t) for t in final_tasks(tci - 1)]
                if tci == TCH - 1:
                    # last chunk: run the pair-0 contraction half of its
                    # output projection during pair-1 attention
                    bg += [
                        (2 * tci, t)
                        for t in final_tasks(tci, ccs=(0,), dst=y2_d)
                    ]
                bg_done = 0
                bg_total = len(bg)
                slots = 2 * 2 * 2 * njp  # pairs x h01 x jp, x4 slot increment
                slot = 0

                def norms_pending_upto(c):
                    return any(ch <= c for ch, _ in pending_norm)

                for pair in range(2):
                    qc = qcat[pair]
                    o_acs = [
                        psacc.tile([128, 512], F32, tag="oacc", name="oacc")
                        for _ in range(2)
                    ]
                    DEPTH = 3
                    pvq = []  # (jp, h01, p_tile, win) awaiting PV emission

                    def emit_pv(jp, h01, p_t, win, o_acs=o_acs, njp=njp):
                        if jp == 0:
                            for i in range(2):
                                nc.tensor.matmul(
                                    o_acs[h01][0:65, :],
                                    v01_sb[:, i, 0:65],
                                    p_t[:, i, :],
                                    start=(i == 0),
                                    stop=False,
                                )
                        else:
                            nc.tensor.matmul(
                                o_acs[h01][0:65, win:512],
                                v8_sb[:, 2 * jp : 2 * jp + 2, 0:65],
                                p_t[:, :, win:512],
                                start=False,
                                stop=(jp == njp - 1),
                                perf_mode=DR,
                            )

                    for jp in range(njp):
                        win = 256 if jp == 2 * tci + 1 else 0
                        diag = jp in (2 * tci, 2 * tci + 1)
                        for h01 in range(2):
                            s_ps = pss.tile([128, 2, 512], F32, tag="s", name="s")
                            for i in range(2):
                                blk = 2 * jp + i
                                nc.tensor.matmul(
                                    s_ps[:, i, win:512],
                                    qkvT[2][0:64, blk * 128 : (blk + 1) * 128],
                                    qc[:, h01, tsl0 + win : tsl0 + 512],
                                    start=True,
                                    stop=True,
                                )
                            pdt = BF16 if jp == 0 else F8E5
                            p_t = spp.tile([128, 2, 512], pdt, tag="p", name="p")
                            nc.scalar.activation(
                                p_t[:, :, win:512], s_ps[:, :, win:512], AF.Exp
                            )
                            if diag:
                                nc.gpsimd.tensor_mul(
                                    p_t[:, :, win : win + 256],
                                    p_t[:, :, win : win + 256],
                                    mask_sb[:] if jp == 0 else mask8_sb[:],
                                )
                            pvq.append((jp, h01, p_t, win))
                            if len(pvq) > DEPTH:
                                emit_pv(*pvq.pop(0))
                        if pending_norm:
                            pending_norm.pop(0)[1]()
                        slot += 4
                        due = bg_total * min(slot, slots) // slots
                        while bg_done < due:
                            gate, fn = bg[bg_done]
                            if gate is not None and norms_pending_upto(gate):
                                break
                            fn()
                            bg_done += 1

                    for args in pvq:
                        emit_pv(*args)
                    for h01 in range(2):
                        g = pair * 2 + h01
                        pending_norm.append(
                            (2 * tci + pair, mk_norm(g, tci, o_acs[h01]))
                        )

                while bg_done < bg_total:
                    gate, fn = bg[bg_done]
                    if gate is not None and norms_pending_upto(gate):
                        # flush enough pending norms to unblock
                        while norms_pending_upto(gate):
                            pending_norm.pop(0)[1]()
                    fn()
                    bg_done += 1

            for _, fn in pending_norm:
                fn()
            for t in final_tasks(TCH - 1, ccs=(1,)):
                t()

    nc.compile()
    return nc


def host_shard(inputs):
    """Build the 8 per-core input maps from full inputs."""
    x = np.ascontiguousarray(np.asarray(inputs["input"], dtype=np.float32))
    W = np.asarray(inputs["W_attn"], dtype=np.float32)
    bb = np.asarray(inputs["b_attn"], dtype=np.float32)
    Wf = np.asarray(inputs["W_final"], dtype=np.float32)

    half = HS // 2
    inv_freq = MAX_PERIOD ** (-np.arange(half, dtype=np.float32) / half)
    ang = np.arange(T, dtype=np.float32)[:, None] * inv_freq  # (T, 32)
    sin_t = np.sin(ang).astype(np.float32)
    cos_t = np.cos(ang).astype(np.float32)
    cosT = np.repeat(cos_t.T, 2, axis=0)  # (64, T): row d -> cos(t*f[d//2])
    sgn = np.where(np.arange(HS) % 2 == 0, -1.0, 1.0).astype(np.float32)
    sinT = np.repeat(sin_t.T, 2, axis=0) * sgn[:, None]
    cos128 = np.ascontiguousarray(np.concatenate([cosT, cosT], axis=0))
    sin128 = np.ascontiguousarray(np.concatenate([sinT, sinT], axis=0))

    perm = np.zeros((128, 128), np.float32)
    idx = np.arange(128)
    perm[idx ^ 1, idx] = 1.0
    eye64 = np.zeros((128, 64), np.float32)
    eye64[64:128, :] = np.eye(64, dtype=np.float32)

    # 0/1 keep-mask for the two diagonal key-blocks of each chunk, applied on
    # the first 256 computed queries of a (block-pair, window):
    #   slot 0 (earlier block):  u<128: keep kk<=u ; u in [128,256): keep all
    #   slot 1 (later block):    u<128: keep none  ; u in [128,256): keep kk<=u-128
    kk = np.arange(128)[:, None]
    u = np.arange(256)[None, :]
    m0 = np.where(u < 128, (kk <= u), True)
    m1 = np.where(u < 128, False, (kk <= u - 128))
    mask01 = np.ascontiguousarray(
        np.stack([m0, m1], axis=1).astype(np.float32)
    )  # (128, 2, 256)

    ones64 = np.ones((128, 64), np.float32)

    in_maps = []
    for cid in range(8):
        b, h = cid // 4, cid % 4
        qcols = np.concatenate(
            [np.arange(g * 256 + h * 64, g * 256 + h * 64 + 64) for g in range(G)]
        )
        kcols = np.arange(1024 + h * 64, 1024 + h * 64 + 64)
        vcols = np.arange(1280 + h * 64, 1280 + h * 64 + 64)
        cols = np.concatenate([qcols, kcols, vcols])
        w_loc = W[:, cols].copy()
        b_loc = bb[cols].copy()
        w_loc[:, :256] *= SCALE
        b_loc[:256] *= SCALE
        b_loc_m = np.ascontiguousarray(b_loc.reshape(3, 128).T)  # (128, 3)

        rows = np.concatenate(
            [np.arange(g * 256 + h * 64, g * 256 + h * 64 + 64) for g in range(G)]
        )
        wf_loc = np.ascontiguousarray(Wf[rows, :])  # (256, 1024)

        in_maps.append(
            {
                "xT": np.ascontiguousarray(x[b].T).astype(ml_dtypes.bfloat16),
                "w_qkv": w_loc.astype(ml_dtypes.bfloat16),
                "b_loc": b_loc_m,
                "cosT": cos128.astype(ml_dtypes.bfloat16),
                "sinT": sin128.astype(ml_dtypes.bfloat16),
                "perm": perm.astype(ml_dtypes.bfloat16),
                "eye64": eye64.astype(ml_dtypes.bfloat16),
                "mask01": mask01.astype(ml_dtypes.bfloat16),
                "wf": wf_loc.astype(ml_dtypes.bfloat16),
                "onesd": ones64.astype(ml_dtypes.bfloat16),
            }
        )
    return in_maps


def host_unshard(results, b_final):
    """Sum the 4 per-h partial yT per batch (folding in the last chunk's
    early contraction-half partial y2T), add bias, transpose back."""
    out = np.empty((B, T, C), np.float32)
    for b in range(B):
        acc = results[b * 4]["yT"].astype(np.float32)
        acc[:, 3 * 512 :] += results[b * 4]["y2T"].astype(np.float32)
        for h in range(1, 4):
            acc = acc + results[b * 4 + h]["yT"]
            acc[:, 3 * 512 :] += results[b * 4 + h]["y2T"].astype(np.float32)
        out[b] = acc.T + b_final[None, :]
    return out


_NC_CACHE = None


def _get_nc():
    global _NC_CACHE
    if _NC_CACHE is None:
        _NC_CACHE = build_nc()
    return _NC_CACHE


def kernel(**inputs):
    nc = _get_nc()
    in_maps = host_shard(inputs)
    res = run_bass_kernel_spmd(nc, in_maps, core_ids=list(range(8)))
    return host_unshard(res.results, np.asarray(inputs["b_final"], dtype=np.float32))
